# revision 17
# baseline (speedup 1.0000x reference)
"""Causal self-attention (B=4, T=2048, D=1024, H=16) on 8 NeuronCores.

Sharding: core c handles batch b=c//2 and head-group hg=c%2 (8 of 16 heads).
Per core: column-parallel Wq/Wk/Wv (512 cols), row-parallel Wo (512 rows).
Host sums the two partial outputs per batch and adds bo. No collectives.

All-bf16 compute (f32 PSUM accumulation). On-chip layout (transposed):
  xt [D=1024, T=2048] resident in SBUF (loaded once), qT/kT [128, T] per
  head pair, V natural [tk, 8 heads x (64 dv + 1 ones col)].
  S^T [tk, tq] for BOTH heads of a pair lands in one 2-bank PSUM tile
  [128, 1024] via two concurrent row-tiled matmuls; ONE exp per k-tile
  covers both heads (no max subtraction -- scores are O(4), exp safe).
  Causal masking: diagonal k-tiles narrow matmul/exp to the valid column
  range; the triangular boundary is zeroed AFTER exp by a [128,128] bf16
  multiply (same result as -inf masking, off the scalar-engine path).
  PV matmul out^T[dv,tq] = V_aug.T @ expS^T; the ones column yields
  sumexp for free. Sumexp rows are gathered cross-partition into one
  [8,512] tile per pair (DVE copy + tiny SBUF->SBUF DMA) so reciprocal
  runs batched (the per-row [1,512] reciprocal was 3.3us each).
  Normalization: K=1 broadcast matmul of 1/sumexp + in-place DVE mul
  into oat; final projection consumes oat directly as lhsT.

PE (HAM) warmth: the scalar engine's exp is the attention-phase pace
setter, so next pair's Q/K projection chains and deferred V-projection
tiles are emitted interleaved between attention k-tiles to keep the
tensor engine dense (K=8/8 clock).
"""

import os
from contextlib import ExitStack

import ml_dtypes
import numpy as np

import concourse.bacc as bacc
import concourse.mybir as mybir
import concourse.tile as tile
from concourse.bass_utils import run_bass_kernel_spmd

B, T, D, H, DK = 4, 2048, 1024, 16, 64
HL = 8  # heads per core
CD = HL * DK  # 512 local channels
NP = 128  # partitions
QB = 512  # query block
NDC = D // NP  # 8 din chunks
NTT = T // NP  # 16 t-tiles
NTB = T // QB  # 4 t-blocks
NPAIR = HL // 2  # 4 head pairs
NV_UP = 5  # V tiles emitted upfront (rest injected into attention 0)
F32 = mybir.dt.float32
BF16 = mybir.dt.bfloat16
Exp = mybir.ActivationFunctionType.Exp

_CACHE: dict = {}


def _build_nc():
    nc = bacc.Bacc("TRN2", target_bir_lowering=False, debug=False)
    xt = nc.dram_tensor("xt", [D, T], BF16, kind="ExternalInput")
    wq = nc.dram_tensor("wq", [D, CD], BF16, kind="ExternalInput")
    wk = nc.dram_tensor("wk", [D, CD], BF16, kind="ExternalInput")
    wv = nc.dram_tensor("wv", [D, CD], BF16, kind="ExternalInput")
    wo = nc.dram_tensor("wo", [CD, D], BF16, kind="ExternalInput")
    bqc = nc.dram_tensor("bqc", [NP, NPAIR], F32, kind="ExternalInput")
    bkc = nc.dram_tensor("bkc", [NP, NPAIR], F32, kind="ExternalInput")
    bvr = nc.dram_tensor("bvr", [1, CD], BF16, kind="ExternalInput")
    trid = nc.dram_tensor("trid", [NP, NP], BF16, kind="ExternalInput")
    y = nc.dram_tensor("y", [T, D], F32, kind="ExternalOutput")

    with tile.TileContext(nc) as tc, ExitStack() as ctx:
        _body(nc, tc, ctx, xt, wq, wk, wv, wo, bqc, bkc, bvr, trid, y)
    nc.compile()
    return nc


def _body(nc, tc, ctx, xt, wq, wk, wv, wo, bqc, bkc, bvr, trid, y):
    const = ctx.enter_context(tc.tile_pool(name="const", bufs=1))
    xtp = ctx.enter_context(tc.tile_pool(name="xtp", bufs=1))
    wp = ctx.enter_context(tc.tile_pool(name="wp", bufs=1))
    vpool = ctx.enter_context(tc.tile_pool(name="v", bufs=1))
    oatp = ctx.enter_context(tc.tile_pool(name="oat", bufs=1))
    qkp = ctx.enter_context(tc.tile_pool(name="qk", bufs=2))
    expp = ctx.enter_context(tc.tile_pool(name="exp", bufs=3))
    smallp = ctx.enter_context(tc.tile_pool(name="small", bufs=2))
    # PSUM: proj 2 + score (2-bank x 2) 4 + pv0 1 + pv1 1 = 8 banks
    projps = ctx.enter_context(tc.tile_pool(name="projps", bufs=2, space="PSUM"))
    scoreps = ctx.enter_context(tc.tile_pool(name="scoreps", bufs=2, space="PSUM"))
    pvps = ctx.enter_context(tc.tile_pool(name="pvps", bufs=1, space="PSUM"))

    # ---- constants ----
    tri2_sb = const.tile([NP, 2, NP], BF16, tag="tri")
    for g in range(2):
        nc.sync.dma_start(tri2_sb[:, g, :], trid[:])
    bq_sb = const.tile([NP, NPAIR], F32, tag="bq")
    nc.sync.dma_start(bq_sb[:], bqc[:])
    bk_sb = const.tile([NP, NPAIR], F32, tag="bk")
    nc.sync.dma_start(bk_sb[:], bkc[:])
    bv_sb = const.tile([1, CD], BF16, tag="bv")
    nc.sync.dma_start(bv_sb[:], bvr[:])
    ones_t = const.tile([1, NP], BF16, tag="onest")
    nc.vector.memset(ones_t[:], 1.0)
    ones_f = const.tile([1, DK], BF16, tag="onesf")
    nc.vector.memset(ones_f[:], 1.0)
    # warm up the exp table set early (one-time ~2.7us load)
    warm = const.tile([1, 2], F32, tag="warm")
    nc.vector.memset(warm[:], 0.0)
    nc.scalar.activation(warm[:], warm[:], Exp)

    # ---- resident inputs (d-interleaved so V proj / pair-0 proj can
    # start as soon as their first chunks land) ----
    xt_sb = xtp.tile([NP, NDC, T], BF16, tag="xt")
    wv_sb = wp.tile([NP, NDC, CD], BF16, tag="wv")
    wq_sb = wp.tile([NP, NDC, CD], BF16, tag="wq")
    wk_sb = wp.tile([NP, NDC, CD], BF16, tag="wk")
    wo_sb = wp.tile([NP, NPAIR, D], BF16, tag="wo")
    # first T-half of xt + wv unblock the upfront V tiles; pair-0 Wq/Wk
    # unblock its projections; everything else arrives behind them
    TH = T // 2
    for d in range(NDC):
        nc.sync.dma_start(xt_sb[:, d, 0:TH], xt[d * NP : (d + 1) * NP, 0:TH])
        nc.sync.dma_start(wv_sb[:, d, :], wv[d * NP : (d + 1) * NP, :])
    for d in range(NDC):
        nc.sync.dma_start(wq_sb[:, d, 0:NP], wq[d * NP : (d + 1) * NP, 0:NP])
        nc.sync.dma_start(wk_sb[:, d, 0:NP], wk[d * NP : (d + 1) * NP, 0:NP])
        nc.sync.dma_start(xt_sb[:, d, TH:T], xt[d * NP : (d + 1) * NP, TH:T])
    for c in range(1, NPAIR):
        for d in range(NDC):
            nc.sync.dma_start(
                wq_sb[:, d, c * NP : (c + 1) * NP],
                wq[d * NP : (d + 1) * NP, c * NP : (c + 1) * NP],
            )
            nc.sync.dma_start(
                wk_sb[:, d, c * NP : (c + 1) * NP],
                wk[d * NP : (d + 1) * NP, c * NP : (c + 1) * NP],
            )
    for c in range(NPAIR):
        nc.sync.dma_start(wo_sb[:, c, :], wo[c * NP : (c + 1) * NP, :])

    v_sb = [
        vpool.tile([NP, HL, DK + 1], BF16, tag=f"v{tt}", name=f"v{tt}")
        for tt in range(NTT)
    ]
    oat = [
        oatp.tile([NP, T], BF16, tag=f"oat{c}", name=f"oat{c}") for c in range(NPAIR)
    ]

    # ---- deferred-work emitters (injected between attention k-tiles) ----
    def emit_v(tt):
        ps = projps.tile([NP, CD], F32, tag="proj", name="psv")
        for d in range(NDC):
            nc.tensor.matmul(
                ps[:],
                xt_sb[:, d, tt * NP : (tt + 1) * NP],
                wv_sb[:, d, :],
                start=(d == 0),
                stop=False,
            )
        # bias along free dim via K=1 rank-1 update
        nc.tensor.matmul(ps[:], ones_t[0:1, 0:NP], bv_sb[:], start=False, stop=True)
        vt = v_sb[tt]
        nc.vector.memset(vt[:, :, DK : DK + 1], 1.0)
        nc.vector.tensor_copy(vt[:, :, 0:DK], ps.rearrange("p (h k) -> p h k", h=HL))

    def emit_proj(dst, w_sb, b_sb, c, tb):
        ps = projps.tile([NP, QB], F32, tag="proj", name="psp")
        for d in range(NDC):
            nc.tensor.matmul(
                ps[:],
                w_sb[:, d, c * NP : (c + 1) * NP],
                xt_sb[:, d, tb * QB : (tb + 1) * QB],
                start=(d == 0),
                stop=(d == NDC - 1),
            )
        nc.vector.tensor_scalar_add(
            dst[:, tb * QB : (tb + 1) * QB], ps[:], b_sb[:, c : c + 1]
        )

    def proj_tasks(c, qt, kt, tbs=range(NTB)):
        for tb in tbs:
            yield lambda tb=tb: emit_proj(qt, wq_sb, bq_sb, c, tb)
            yield lambda tb=tb: emit_proj(kt, wk_sb, bk_sb, c, tb)

    def final_task(tt, dh):
        ps = projps.tile([NP, QB], F32, tag="proj", name="psy")
        for cc in range(NPAIR):
            nc.tensor.matmul(
                ps[:],
                oat[cc][:, tt * NP : (tt + 1) * NP],
                wo_sb[:, cc, dh * QB : (dh + 1) * QB],
                start=(cc == 0),
                stop=(cc == NPAIR - 1),
            )
        ystage = smallp.tile([NP, QB], F32, tag="ystage", bufs=4, name="ystage")
        if (tt + dh) % 2 == 0:
            nc.vector.tensor_copy(ystage[:], ps[:])
        else:
            nc.scalar.copy(ystage[:], ps[:])
        eng = nc.sync if (tt + dh) % 2 == 0 else nc.gpsimd
        eng.dma_start(y[tt * NP : (tt + 1) * NP, dh * QB : (dh + 1) * QB], ystage[:])

    def norm_task(c, qb, se_q):
        # sumexp arrives partition-folded [128, 8]: reciprocal costs 8
        # elems/lane. Unfold to one partition-0 row, broadcast across 128
        # dv partitions via two concurrent col-tiled K=1 matmuls, then
        # normalize both heads' oat slice with a single in-place mul.
        rcp = smallp.tile([NP, 8], F32, tag="rcp", name="rcp")
        nc.vector.reciprocal(rcp[:], se_q[:])
        rcpb = smallp.tile([NP, 8], BF16, tag="rcpb", name="rcpb")
        nc.vector.tensor_copy(rcpb[:], rcp[:])
        rcprow = smallp.tile([1, 2 * QB], BF16, tag="rcprow", name="rcprow")
        for h in range(2):
            nc.gpsimd.dma_start(
                rcprow[0:1, h * QB : (h + 1) * QB], rcpb[:, h * 4 : (h + 1) * 4]
            )
        bc = projps.tile([NP, QB], F32, tag="proj", name="bc")
        for h in range(2):
            nc.tensor.matmul(
                bc[64 * h : 64 * h + 64, :],
                ones_f[0:1, 0:DK],
                rcprow[0:1, h * QB : (h + 1) * QB],
                start=True, stop=True,
                tile_position=(0, 64 * h),
            )
        sl = oat[c][:, qb * QB : (qb + 1) * QB]
        nc.vector.tensor_mul(sl, sl, bc[:])

    # ---- upfront: first V tiles + pair-0 first-half projections (all
    # depend only on the first T-half of xt) ----
    for tt in range(NV_UP):
        emit_v(tt)
    qt_c = qkp.tile([NP, T], BF16, tag="qt", name="qt0")
    kt_c = qkp.tile([NP, T], BF16, tag="kt", name="kt0")
    for t in proj_tasks(0, qt_c, kt_c, tbs=range(2)):
        t()

    # ---- per head-pair attention with injected deferred work ----
    pending = list(proj_tasks(0, qt_c, kt_c, tbs=range(2, NTB)))
    for c in range(NPAIR):
        if c + 1 < NPAIR:
            qt_n = qkp.tile([NP, T], BF16, tag="qt", name=f"qt{c + 1}")
            kt_n = qkp.tile([NP, T], BF16, tag="kt", name=f"kt{c + 1}")
            if c == 0:
                pending += [lambda tt=tt: emit_v(tt) for tt in range(NV_UP, NTT)]
            pending += list(proj_tasks(c + 1, qt_n, kt_n))
        else:
            qt_n = kt_n = None

        for qb in range(NTB):
            nkt = 4 * qb + 4
            ng = nkt // 4  # exp groups of 4 k-tiles
            pv = [
                pvps.tile([DK + 1, QB], F32, tag=f"pv{h}", name=f"pv{h}")
                for h in range(2)
            ]

            def s_block(g, qb=qb):
                # scores for 4 k-tiles; each tile cast PSUM->SBUF right
                # away (valid columns only) so one exp covers the group
                scg = expp.tile([NP, 4, 2 * QB], BF16, tag="sc", name="scg")
                for j in range(4):
                    kti = 4 * g + j
                    if pending:
                        pending.pop(0)()
                    di = kti - 4 * qb
                    o = max(di, 0) * NP
                    sp = scoreps.tile([NP, 2 * QB], F32, tag="s", name="sp")
                    for h in range(2):
                        nc.tensor.matmul(
                            sp[:, h * QB + o : (h + 1) * QB],
                            kt_c[64 * h : 64 * h + 64, kti * NP : (kti + 1) * NP],
                            qt_c[64 * h : 64 * h + 64, qb * QB + o : (qb + 1) * QB],
                            start=True,
                            stop=True,
                            tile_position=(64 * h, 0),
                        )
                    spv = sp.rearrange("p (a q) -> p a q", a=2)[:, :, o:QB]
                    scv = scg[:, j, :].rearrange("p (a q) -> p a q", a=2)[:, :, o:QB]
                    nc.vector.tensor_copy(scv, spv)
                return scg

            def pv_block(g, et_g, qb=qb):
                for j in range(4):
                    kti = 4 * g + j
                    di = kti - 4 * qb
                    o = max(di, 0) * NP
                    if di >= 0:
                        ev = et_g[:, j, :].rearrange("p (a q) -> p a q", a=2)[
                            :, :, o : o + NP
                        ]
                        nc.vector.tensor_mul(ev, ev, tri2_sb[:])
                    for h in range(2):
                        nc.tensor.matmul(
                            pv[h][:, o:QB],
                            v_sb[kti][:, 2 * c + h, :],
                            et_g[:, j, h * QB + o : (h + 1) * QB],
                            start=(kti == 0),
                            stop=(kti == nkt - 1),
                        )

            scg = s_block(0)
            for g in range(ng):
                et_g = expp.tile([NP, 4, 2 * QB], BF16, tag="e", name="etg")
                nc.scalar.activation(et_g[:], scg[:], Exp, scale=0.125)
                if g + 1 < ng:
                    scg = s_block(g + 1)
                pv_block(g, et_g)
            se_q = smallp.tile([NP, 8], F32, tag="se", bufs=3, name=f"se{c}_{qb}")
            for h in range(2):
                # unnormalized out^T -> oat; sumexp row -> se_q partition-
                # folded [1,512]->[128,4] (same-partition staging copy +
                # cross-partition SBUF DMA off the sync queue)
                nc.vector.tensor_copy(
                    oat[c][64 * h : 64 * h + 64, qb * QB : (qb + 1) * QB],
                    pv[h][0:DK, :],
                )
                serow = smallp.tile([DK + 1, QB], F32, tag="serow", name="serow")
                nc.vector.tensor_copy(serow[DK : DK + 1, :], pv[h][DK : DK + 1, :])
                nc.gpsimd.dma_start(
                    se_q[:, h * 4 : (h + 1) * 4], serow[DK : DK + 1, :]
                )
            pending.append(lambda c=c, qb=qb, se_q=se_q: norm_task(c, qb, se_q))
            if c == NPAIR - 1:
                # final projection chunks for this qb's t-range become
                # injectable as soon as this qb's normalization runs
                # (earlier pairs' oat is long since normalized)
                for tt in range(4 * qb, 4 * qb + 4):
                    for dh in range(2):
                        pending.append(lambda tt=tt, dh=dh: final_task(tt, dh))
        qt_c, kt_c = qt_n, kt_n

    while pending:  # pair 3's tail: last normalizations + final chunks
        pending.pop(0)()


def _install_ntff_hook_shim():
    """The agent image's antenv lacks axon_hooks, so trace=True under axon
    degrades. Provide the missing module and register the ctypes NTFF hook
    from trn_agent_boot. Best-effort: failures just mean no trace."""
    try:
        import sys
        import types

        if "antenv.axon_hooks" not in sys.modules:
            mod = types.ModuleType("antenv.axon_hooks")
            mod._hook = None
            mod.set_axon_ntff_profile_hook = lambda h: setattr(mod, "_hook", h)
            mod.get_axon_ntff_profile_hook = lambda: mod._hook
            sys.modules["antenv.axon_hooks"] = mod
            import antenv

            antenv.axon_hooks = mod
        from antenv.axon_hooks import (
            get_axon_ntff_profile_hook,
            set_axon_ntff_profile_hook,
        )

        if get_axon_ntff_profile_hook() is None:
            from trn_agent_boot.trn_boot import _ntff_profile_via_ctypes

            hook = _ntff_profile_via_ctypes("/opt/axon/libaxon_pjrt.so")
            if hook is not None:
                set_axon_ntff_profile_hook(hook)
    except Exception as e:  # noqa: BLE001
        print(f"ntff hook shim failed ({e}); running without trace")


def _bf(a: np.ndarray) -> np.ndarray:
    return np.ascontiguousarray(a, dtype=np.float32).astype(ml_dtypes.bfloat16)


def _make_tri() -> np.ndarray:
    # tri[r, j] = 1 if j >= r else 0 (valid region of a boundary tile)
    r = np.arange(NP)[:, None]
    j = np.arange(NP)[None, :]
    return (j >= r).astype(ml_dtypes.bfloat16)


def kernel(x, Wq, bq, Wk, bk, Wv, bv, Wo, bo):
    x = np.ascontiguousarray(np.asarray(x, dtype=np.float32))
    Wq, bq = np.asarray(Wq, np.float32), np.asarray(bq, np.float32)
    Wk, bk = np.asarray(Wk, np.float32), np.asarray(bk, np.float32)
    Wv, bv = np.asarray(Wv, np.float32), np.asarray(bv, np.float32)
    Wo, bo = np.asarray(Wo, np.float32), np.asarray(bo, np.float32)

    if "nc" not in _CACHE:
        _CACHE["nc"] = _build_nc()
    nc = _CACHE["nc"]

    tri = _make_tri()
    in_maps = []
    for core in range(8):
        b, hg = core // 2, core % 2
        cs = slice(hg * CD, (hg + 1) * CD)
        in_maps.append(
            {
                "xt": _bf(x[b].T),
                "wq": _bf(Wq[:, cs]),
                "wk": _bf(Wk[:, cs]),
                "wv": _bf(Wv[:, cs]),
                "wo": _bf(Wo[cs, :]),
                "bqc": np.ascontiguousarray(bq[cs].reshape(NPAIR, NP).T),
                "bkc": np.ascontiguousarray(bk[cs].reshape(NPAIR, NP).T),
                "bvr": _bf(bv[cs].reshape(1, CD)),
                "trid": tri,
            }
        )

    trace = bool(os.environ.get("KERNEL_TRACE"))
    if trace:
        _install_ntff_hook_shim()
    res = run_bass_kernel_spmd(nc, in_maps, core_ids=list(range(8)), trace=trace)
    _CACHE["last_results"] = res

    out = np.empty((B, T, D), dtype=np.float32)
    for b in range(B):
        out[b] = res.results[2 * b]["y"] + res.results[2 * b + 1]["y"] + bo
    return out


# revision 18
# speedup vs baseline: 1.1799x; 1.1799x over previous
"""Causal self-attention (B=4, T=2048, D=1024, H=16) on 8 NeuronCores.

Sharding: core c handles batch b=c//2 and head-group hg=c%2 (8 of 16 heads).
Per core: column-parallel Wq/Wk/Wv (512 cols), row-parallel Wo (512 rows).
Host sums the two partial outputs per batch and adds bo. No collectives.

All-bf16 compute (f32 PSUM accumulation). On-chip layout (transposed):
  xt [D=1024, T=2048] resident in SBUF (loaded once), qT/kT [128, T] per
  head pair, V natural [tk, 8 heads x (64 dv + 1 ones col)].
  S^T [tk, tq] for BOTH heads of a pair lands in one 2-bank PSUM tile
  [128, 1024] via two concurrent row-tiled matmuls; ONE exp per k-tile
  covers both heads (no max subtraction -- scores are O(4), exp safe).
  Causal masking: diagonal k-tiles narrow matmul/exp to the valid column
  range; the triangular boundary is zeroed AFTER exp by a [128,128] bf16
  multiply (same result as -inf masking, off the scalar-engine path).
  PV matmul out^T[dv,tq] = V_aug.T @ expS^T; the ones column yields
  sumexp for free. Sumexp rows are gathered cross-partition into one
  [8,512] tile per pair (DVE copy + tiny SBUF->SBUF DMA) so reciprocal
  runs batched (the per-row [1,512] reciprocal was 3.3us each).
  Normalization: K=1 broadcast matmul of 1/sumexp + in-place DVE mul
  into oat; final projection consumes oat directly as lhsT.

PE (HAM) warmth: the scalar engine's exp is the attention-phase pace
setter, so next pair's Q/K projection chains and deferred V-projection
tiles are emitted interleaved between attention k-tiles to keep the
tensor engine dense (K=8/8 clock).
"""

import os
from contextlib import ExitStack

import ml_dtypes
import numpy as np

import concourse.bacc as bacc
import concourse.mybir as mybir
import concourse.tile as tile
from concourse.bass_utils import run_bass_kernel_spmd

B, T, D, H, DK = 4, 2048, 1024, 16, 64
HL = 8  # heads per core
CD = HL * DK  # 512 local channels
NP = 128  # partitions
QB = 512  # query block
NDC = D // NP  # 8 din chunks
NTT = T // NP  # 16 t-tiles
NTB = T // QB  # 4 t-blocks
NPAIR = HL // 2  # 4 head pairs
NV_UP = 5  # V tiles emitted upfront (rest injected into attention 0)
F32 = mybir.dt.float32
BF16 = mybir.dt.bfloat16
Exp = mybir.ActivationFunctionType.Exp

_CACHE: dict = {}


def _build_nc():
    nc = bacc.Bacc("TRN2", target_bir_lowering=False, debug=False)
    xt = nc.dram_tensor("xt", [D, T], BF16, kind="ExternalInput")
    wq = nc.dram_tensor("wq", [D, CD], BF16, kind="ExternalInput")
    wk = nc.dram_tensor("wk", [D, CD], BF16, kind="ExternalInput")
    wv = nc.dram_tensor("wv", [D, CD], BF16, kind="ExternalInput")
    wo = nc.dram_tensor("wo", [CD, D], BF16, kind="ExternalInput")
    bqc = nc.dram_tensor("bqc", [NP, NPAIR], F32, kind="ExternalInput")
    bkc = nc.dram_tensor("bkc", [NP, NPAIR], F32, kind="ExternalInput")
    bvr = nc.dram_tensor("bvr", [1, CD], BF16, kind="ExternalInput")
    trid = nc.dram_tensor("trid", [NP, NP], BF16, kind="ExternalInput")
    y = nc.dram_tensor("y", [T, D], F32, kind="ExternalOutput")

    with tile.TileContext(nc) as tc, ExitStack() as ctx:
        _body(nc, tc, ctx, xt, wq, wk, wv, wo, bqc, bkc, bvr, trid, y)
    nc.compile()
    return nc


def _body(nc, tc, ctx, xt, wq, wk, wv, wo, bqc, bkc, bvr, trid, y):
    const = ctx.enter_context(tc.tile_pool(name="const", bufs=1))
    xtp = ctx.enter_context(tc.tile_pool(name="xtp", bufs=1))
    wp = ctx.enter_context(tc.tile_pool(name="wp", bufs=1))
    vpool = ctx.enter_context(tc.tile_pool(name="v", bufs=1))
    oatp = ctx.enter_context(tc.tile_pool(name="oat", bufs=1))
    qkp = ctx.enter_context(tc.tile_pool(name="qk", bufs=2))
    expp = ctx.enter_context(tc.tile_pool(name="exp", bufs=3))
    smallp = ctx.enter_context(tc.tile_pool(name="small", bufs=2))
    # PSUM: proj 2 + score (2-bank x 2) 4 + pv0 1 + pv1 1 = 8 banks
    projps = ctx.enter_context(tc.tile_pool(name="projps", bufs=2, space="PSUM"))
    scoreps = ctx.enter_context(tc.tile_pool(name="scoreps", bufs=2, space="PSUM"))
    pvps = ctx.enter_context(tc.tile_pool(name="pvps", bufs=1, space="PSUM"))

    # ---- constants ----
    tri2_sb = const.tile([NP, 2, NP], BF16, tag="tri")
    for g in range(2):
        nc.sync.dma_start(tri2_sb[:, g, :], trid[:])
    bq_sb = const.tile([NP, NPAIR], F32, tag="bq")
    nc.sync.dma_start(bq_sb[:], bqc[:])
    bk_sb = const.tile([NP, NPAIR], F32, tag="bk")
    nc.sync.dma_start(bk_sb[:], bkc[:])
    bv_sb = const.tile([1, CD], BF16, tag="bv")
    nc.sync.dma_start(bv_sb[:], bvr[:])
    ones_t = const.tile([1, NP], BF16, tag="onest")
    nc.vector.memset(ones_t[:], 1.0)
    ones_f = const.tile([1, DK], BF16, tag="onesf")
    nc.vector.memset(ones_f[:], 1.0)
    # warm up the exp table set early (one-time ~2.7us load)
    warm = const.tile([1, 2], F32, tag="warm")
    nc.vector.memset(warm[:], 0.0)
    nc.scalar.activation(warm[:], warm[:], Exp)

    # ---- resident inputs (d-interleaved so V proj / pair-0 proj can
    # start as soon as their first chunks land) ----
    xt_sb = xtp.tile([NP, NDC, T], BF16, tag="xt")
    wv_sb = wp.tile([NP, NDC, CD], BF16, tag="wv")
    wq_sb = wp.tile([NP, NDC, CD], BF16, tag="wq")
    wk_sb = wp.tile([NP, NDC, CD], BF16, tag="wk")
    wo_sb = wp.tile([NP, NPAIR, D], BF16, tag="wo")
    # first T-half of xt + wv unblock the upfront V tiles; pair-0 Wq/Wk
    # unblock its projections; everything else arrives behind them
    TH = T // 2
    for d in range(NDC):
        nc.sync.dma_start(xt_sb[:, d, 0:TH], xt[d * NP : (d + 1) * NP, 0:TH])
        nc.sync.dma_start(wv_sb[:, d, :], wv[d * NP : (d + 1) * NP, :])
    for d in range(NDC):
        nc.sync.dma_start(wq_sb[:, d, 0:NP], wq[d * NP : (d + 1) * NP, 0:NP])
        nc.sync.dma_start(wk_sb[:, d, 0:NP], wk[d * NP : (d + 1) * NP, 0:NP])
        nc.sync.dma_start(xt_sb[:, d, TH:T], xt[d * NP : (d + 1) * NP, TH:T])
    for c in range(1, NPAIR):
        for d in range(NDC):
            nc.sync.dma_start(
                wq_sb[:, d, c * NP : (c + 1) * NP],
                wq[d * NP : (d + 1) * NP, c * NP : (c + 1) * NP],
            )
            nc.sync.dma_start(
                wk_sb[:, d, c * NP : (c + 1) * NP],
                wk[d * NP : (d + 1) * NP, c * NP : (c + 1) * NP],
            )
    for c in range(NPAIR):
        nc.sync.dma_start(wo_sb[:, c, :], wo[c * NP : (c + 1) * NP, :])

    v_sb = [
        vpool.tile([NP, HL, DK + 1], BF16, tag=f"v{tt}", name=f"v{tt}")
        for tt in range(NTT)
    ]
    oat = [
        oatp.tile([NP, T], BF16, tag=f"oat{c}", name=f"oat{c}") for c in range(NPAIR)
    ]

    # ---- deferred-work emitters (injected between attention k-tiles) ----
    def emit_v(tt):
        ps = projps.tile([NP, CD], F32, tag="proj", name="psv")
        for d in range(NDC):
            nc.tensor.matmul(
                ps[:],
                xt_sb[:, d, tt * NP : (tt + 1) * NP],
                wv_sb[:, d, :],
                start=(d == 0),
                stop=False,
            )
        # bias along free dim via K=1 rank-1 update
        nc.tensor.matmul(ps[:], ones_t[0:1, 0:NP], bv_sb[:], start=False, stop=True)
        vt = v_sb[tt]
        nc.vector.memset(vt[:, :, DK : DK + 1], 1.0)
        nc.vector.tensor_copy(vt[:, :, 0:DK], ps.rearrange("p (h k) -> p h k", h=HL))

    def emit_proj(dst, w_sb, b_sb, c, tb):
        ps = projps.tile([NP, QB], F32, tag="proj", name="psp")
        for d in range(NDC):
            nc.tensor.matmul(
                ps[:],
                w_sb[:, d, c * NP : (c + 1) * NP],
                xt_sb[:, d, tb * QB : (tb + 1) * QB],
                start=(d == 0),
                stop=(d == NDC - 1),
            )
        nc.vector.tensor_scalar_add(
            dst[:, tb * QB : (tb + 1) * QB], ps[:], b_sb[:, c : c + 1]
        )

    def proj_tasks(c, qt, kt, tbs=range(NTB)):
        for tb in tbs:
            yield lambda tb=tb: emit_proj(qt, wq_sb, bq_sb, c, tb)
            yield lambda tb=tb: emit_proj(kt, wk_sb, bk_sb, c, tb)

    def final_task(tt, dh):
        ps = projps.tile([NP, QB], F32, tag="proj", name="psy")
        for cc in range(NPAIR):
            nc.tensor.matmul(
                ps[:],
                oat[cc][:, tt * NP : (tt + 1) * NP],
                wo_sb[:, cc, dh * QB : (dh + 1) * QB],
                start=(cc == 0),
                stop=(cc == NPAIR - 1),
            )
        ystage = smallp.tile([NP, QB], F32, tag="ystage", bufs=4, name="ystage")
        if (tt + dh) % 2 == 0:
            nc.vector.tensor_copy(ystage[:], ps[:])
        else:
            nc.scalar.copy(ystage[:], ps[:])
        eng = nc.sync if (tt + dh) % 2 == 0 else nc.gpsimd
        eng.dma_start(y[tt * NP : (tt + 1) * NP, dh * QB : (dh + 1) * QB], ystage[:])

    def norm_task(c, qb, se_q):
        # sumexp arrives partition-folded [128, 8]: reciprocal costs 8
        # elems/lane. Unfold to one partition-0 row, broadcast across 128
        # dv partitions via two concurrent col-tiled K=1 matmuls, then
        # normalize both heads' oat slice with a single in-place mul.
        rcp = smallp.tile([NP, 8], F32, tag="rcp", name="rcp")
        nc.vector.reciprocal(rcp[:], se_q[:])
        rcpb = smallp.tile([NP, 8], BF16, tag="rcpb", name="rcpb")
        nc.vector.tensor_copy(rcpb[:], rcp[:])
        rcprow = smallp.tile([1, 2 * QB], BF16, tag="rcprow", name="rcprow")
        for h in range(2):
            nc.gpsimd.dma_start(
                rcprow[0:1, h * QB : (h + 1) * QB], rcpb[:, h * 4 : (h + 1) * 4]
            )
        bc = projps.tile([NP, QB], F32, tag="proj", name="bc")
        for h in range(2):
            nc.tensor.matmul(
                bc[64 * h : 64 * h + 64, :],
                ones_f[0:1, 0:DK],
                rcprow[0:1, h * QB : (h + 1) * QB],
                start=True, stop=True,
                tile_position=(0, 64 * h),
            )
        sl = oat[c][:, qb * QB : (qb + 1) * QB]
        nc.vector.tensor_mul(sl, sl, bc[:])

    # ---- upfront: first V tiles + pair-0 first-half projections (all
    # depend only on the first T-half of xt) ----
    for tt in range(NV_UP):
        emit_v(tt)
    qt_c = qkp.tile([NP, T], BF16, tag="qt", name="qt0")
    kt_c = qkp.tile([NP, T], BF16, tag="kt", name="kt0")
    for t in proj_tasks(0, qt_c, kt_c, tbs=range(2)):
        t()

    # ---- per head-pair attention with injected deferred work ----
    pending = list(proj_tasks(0, qt_c, kt_c, tbs=range(2, NTB)))
    for c in range(NPAIR):
        if c + 1 < NPAIR:
            qt_n = qkp.tile([NP, T], BF16, tag="qt", name=f"qt{c + 1}")
            kt_n = qkp.tile([NP, T], BF16, tag="kt", name=f"kt{c + 1}")
            if c == 0:
                pending += [lambda tt=tt: emit_v(tt) for tt in range(NV_UP, NTT)]
            pending += list(proj_tasks(c + 1, qt_n, kt_n))
        else:
            qt_n = kt_n = None

        for qb in range(NTB):
            nkt = 4 * qb + 4
            pv = [
                pvps.tile([DK + 1, QB], F32, tag=f"pv{h}", name=f"pv{h}")
                for h in range(2)
            ]
            for kti in range(nkt):
                if pending:
                    pending.pop(0)()
                di = kti - 4 * qb
                o = max(di, 0) * NP
                sp = scoreps.tile([NP, 2 * QB], F32, tag="s", name="sp")
                for h in range(2):
                    nc.tensor.matmul(
                        sp[:, h * QB + o : (h + 1) * QB],
                        kt_c[64 * h : 64 * h + 64, kti * NP : (kti + 1) * NP],
                        qt_c[64 * h : 64 * h + 64, qb * QB + o : (qb + 1) * QB],
                        start=True,
                        stop=True,
                        tile_position=(64 * h, 0),
                    )
                et = expp.tile([NP, 2 * QB], BF16, tag="e", name="et")
                nc.scalar.activation(et[:, o : 2 * QB], sp[:, o : 2 * QB], Exp, scale=0.125)
                if di >= 0:
                    ev = et.rearrange("p (g q) -> p g q", g=2)[:, :, o : o + NP]
                    nc.vector.tensor_mul(ev, ev, tri2_sb[:])
                for h in range(2):
                    nc.tensor.matmul(
                        pv[h][:, o:QB],
                        v_sb[kti][:, 2 * c + h, :],
                        et[:, h * QB + o : (h + 1) * QB],
                        start=(kti == 0),
                        stop=(kti == nkt - 1),
                    )
            se_q = smallp.tile([NP, 8], F32, tag="se", bufs=3, name=f"se{c}_{qb}")
            for h in range(2):
                # unnormalized out^T -> oat; sumexp row -> se_q partition-
                # folded [1,512]->[128,4] (same-partition staging copy +
                # cross-partition SBUF DMA off the sync queue)
                nc.vector.tensor_copy(
                    oat[c][64 * h : 64 * h + 64, qb * QB : (qb + 1) * QB],
                    pv[h][0:DK, :],
                )
                serow = smallp.tile([DK + 1, QB], F32, tag="serow", name="serow")
                nc.vector.tensor_copy(serow[DK : DK + 1, :], pv[h][DK : DK + 1, :])
                nc.gpsimd.dma_start(
                    se_q[:, h * 4 : (h + 1) * 4], serow[DK : DK + 1, :]
                )
            pending.append(lambda c=c, qb=qb, se_q=se_q: norm_task(c, qb, se_q))
            if c == NPAIR - 1:
                # final projection chunks for this qb's t-range become
                # injectable as soon as this qb's normalization runs
                # (earlier pairs' oat is long since normalized)
                for tt in range(4 * qb, 4 * qb + 4):
                    for dh in range(2):
                        pending.append(lambda tt=tt, dh=dh: final_task(tt, dh))
        qt_c, kt_c = qt_n, kt_n

    while pending:  # pair 3's tail: last normalizations + final chunks
        pending.pop(0)()


def _install_ntff_hook_shim():
    """The agent image's antenv lacks axon_hooks, so trace=True under axon
    degrades. Provide the missing module and register the ctypes NTFF hook
    from trn_agent_boot. Best-effort: failures just mean no trace."""
    try:
        import sys
        import types

        if "antenv.axon_hooks" not in sys.modules:
            mod = types.ModuleType("antenv.axon_hooks")
            mod._hook = None
            mod.set_axon_ntff_profile_hook = lambda h: setattr(mod, "_hook", h)
            mod.get_axon_ntff_profile_hook = lambda: mod._hook
            sys.modules["antenv.axon_hooks"] = mod
            import antenv

            antenv.axon_hooks = mod
        from antenv.axon_hooks import (
            get_axon_ntff_profile_hook,
            set_axon_ntff_profile_hook,
        )

        if get_axon_ntff_profile_hook() is None:
            from trn_agent_boot.trn_boot import _ntff_profile_via_ctypes

            hook = _ntff_profile_via_ctypes("/opt/axon/libaxon_pjrt.so")
            if hook is not None:
                set_axon_ntff_profile_hook(hook)
    except Exception as e:  # noqa: BLE001
        print(f"ntff hook shim failed ({e}); running without trace")


def _bf(a: np.ndarray) -> np.ndarray:
    return np.ascontiguousarray(a, dtype=np.float32).astype(ml_dtypes.bfloat16)


def _make_tri() -> np.ndarray:
    # tri[r, j] = 1 if j >= r else 0 (valid region of a boundary tile)
    r = np.arange(NP)[:, None]
    j = np.arange(NP)[None, :]
    return (j >= r).astype(ml_dtypes.bfloat16)


def kernel(x, Wq, bq, Wk, bk, Wv, bv, Wo, bo):
    x = np.ascontiguousarray(np.asarray(x, dtype=np.float32))
    Wq, bq = np.asarray(Wq, np.float32), np.asarray(bq, np.float32)
    Wk, bk = np.asarray(Wk, np.float32), np.asarray(bk, np.float32)
    Wv, bv = np.asarray(Wv, np.float32), np.asarray(bv, np.float32)
    Wo, bo = np.asarray(Wo, np.float32), np.asarray(bo, np.float32)

    if "nc" not in _CACHE:
        _CACHE["nc"] = _build_nc()
    nc = _CACHE["nc"]

    tri = _make_tri()
    in_maps = []
    for core in range(8):
        b, hg = core // 2, core % 2
        cs = slice(hg * CD, (hg + 1) * CD)
        in_maps.append(
            {
                "xt": _bf(x[b].T),
                "wq": _bf(Wq[:, cs]),
                "wk": _bf(Wk[:, cs]),
                "wv": _bf(Wv[:, cs]),
                "wo": _bf(Wo[cs, :]),
                "bqc": np.ascontiguousarray(bq[cs].reshape(NPAIR, NP).T),
                "bkc": np.ascontiguousarray(bk[cs].reshape(NPAIR, NP).T),
                "bvr": _bf(bv[cs].reshape(1, CD)),
                "trid": tri,
            }
        )

    trace = bool(os.environ.get("KERNEL_TRACE"))
    if trace:
        _install_ntff_hook_shim()
    res = run_bass_kernel_spmd(nc, in_maps, core_ids=list(range(8)), trace=trace)
    _CACHE["last_results"] = res

    out = np.empty((B, T, D), dtype=np.float32)
    for b in range(B):
        out[b] = res.results[2 * b]["y"] + res.results[2 * b + 1]["y"] + bo
    return out


# revision 22
# speedup vs baseline: 1.2480x; 1.0578x over previous
"""Causal self-attention (B=4, T=2048, D=1024, H=16) on 8 NeuronCores.

Sharding: core c handles batch b=c//2 and head-group hg=c%2 (8 of 16 heads).
Per core: column-parallel Wq/Wk/Wv (512 cols), row-parallel Wo (512 rows).
Host sums the two partial outputs per batch and adds bo. No collectives.

All-bf16 compute (f32 PSUM accumulation). On-chip layout (transposed):
  xt [D=1024, T=2048] resident in SBUF (loaded once), qT/kT [128, T] per
  head pair, V natural [tk, 8 heads x (64 dv + 1 ones col)].
  S^T [tk, tq] for BOTH heads of a pair lands in one 2-bank PSUM tile
  [128, 1024] via two concurrent row-tiled matmuls; ONE exp per k-tile
  covers both heads (no max subtraction -- scores are O(4), exp safe).
  Causal masking: diagonal k-tiles narrow matmul/exp to the valid column
  range; the triangular boundary is zeroed AFTER exp by a [128,128] bf16
  multiply (same result as -inf masking, off the scalar-engine path).
  PV matmul out^T[dv,tq] = V_aug.T @ expS^T; the ones column yields
  sumexp for free. Sumexp rows are gathered cross-partition into one
  [8,512] tile per pair (DVE copy + tiny SBUF->SBUF DMA) so reciprocal
  runs batched (the per-row [1,512] reciprocal was 3.3us each).
  Normalization: K=1 broadcast matmul of 1/sumexp + in-place DVE mul
  into oat; final projection consumes oat directly as lhsT.

PE (HAM) warmth: the scalar engine's exp is the attention-phase pace
setter, so next pair's Q/K projection chains and deferred V-projection
tiles are emitted interleaved between attention k-tiles to keep the
tensor engine dense (K=8/8 clock).
"""

import os
from contextlib import ExitStack

import ml_dtypes
import numpy as np

import concourse.bacc as bacc
import concourse.mybir as mybir
import concourse.tile as tile
from concourse.bass_utils import run_bass_kernel_spmd

B, T, D, H, DK = 4, 2048, 1024, 16, 64
HL = 8  # heads per core
CD = HL * DK  # 512 local channels
NP = 128  # partitions
QB = 512  # query block
NDC = D // NP  # 8 din chunks
NTT = T // NP  # 16 t-tiles
NTB = T // QB  # 4 t-blocks
NPAIR = HL // 2  # 4 head pairs
NV_UP = 5  # V tiles emitted upfront (rest injected into attention 0)
F32 = mybir.dt.float32
BF16 = mybir.dt.bfloat16
Exp = mybir.ActivationFunctionType.Exp

_CACHE: dict = {}


def _build_nc():
    nc = bacc.Bacc("TRN2", target_bir_lowering=False, debug=False)
    xt = nc.dram_tensor("xt", [D, T], BF16, kind="ExternalInput")
    wq = nc.dram_tensor("wq", [D, CD], BF16, kind="ExternalInput")
    wk = nc.dram_tensor("wk", [D, CD], BF16, kind="ExternalInput")
    wv = nc.dram_tensor("wv", [D, CD], BF16, kind="ExternalInput")
    wo = nc.dram_tensor("wo", [CD, D], BF16, kind="ExternalInput")
    bqc = nc.dram_tensor("bqc", [NP, NPAIR], F32, kind="ExternalInput")
    bkc = nc.dram_tensor("bkc", [NP, NPAIR], F32, kind="ExternalInput")
    bvr = nc.dram_tensor("bvr", [1, CD], BF16, kind="ExternalInput")
    trid = nc.dram_tensor("trid", [NP, NP], BF16, kind="ExternalInput")
    y = nc.dram_tensor("y", [T, D], F32, kind="ExternalOutput")

    with tile.TileContext(nc) as tc, ExitStack() as ctx:
        _body(nc, tc, ctx, xt, wq, wk, wv, wo, bqc, bkc, bvr, trid, y)
    nc.compile()
    return nc


def _body(nc, tc, ctx, xt, wq, wk, wv, wo, bqc, bkc, bvr, trid, y):
    const = ctx.enter_context(tc.tile_pool(name="const", bufs=1))
    xtp = ctx.enter_context(tc.tile_pool(name="xtp", bufs=1))
    wp = ctx.enter_context(tc.tile_pool(name="wp", bufs=1))
    vpool = ctx.enter_context(tc.tile_pool(name="v", bufs=1))
    oatp = ctx.enter_context(tc.tile_pool(name="oat", bufs=1))
    qkp = ctx.enter_context(tc.tile_pool(name="qk", bufs=2))
    expp = ctx.enter_context(tc.tile_pool(name="exp", bufs=3))
    smallp = ctx.enter_context(tc.tile_pool(name="small", bufs=2))
    # PSUM: proj 2 + score (2-bank x 2) 4 + pv0 1 + pv1 1 = 8 banks
    projps = ctx.enter_context(tc.tile_pool(name="projps", bufs=2, space="PSUM"))
    scoreps = ctx.enter_context(tc.tile_pool(name="scoreps", bufs=2, space="PSUM"))
    pvps = ctx.enter_context(tc.tile_pool(name="pvps", bufs=1, space="PSUM"))

    # ---- constants ----
    tri2_sb = const.tile([NP, 2, NP], BF16, tag="tri")
    for g in range(2):
        nc.sync.dma_start(tri2_sb[:, g, :], trid[:])
    bq_sb = const.tile([NP, NPAIR], F32, tag="bq")
    nc.sync.dma_start(bq_sb[:], bqc[:])
    bk_sb = const.tile([NP, NPAIR], F32, tag="bk")
    nc.sync.dma_start(bk_sb[:], bkc[:])
    bv_sb = const.tile([1, CD], BF16, tag="bv")
    nc.sync.dma_start(bv_sb[:], bvr[:])
    ones_t = const.tile([1, NP], BF16, tag="onest")
    nc.vector.memset(ones_t[:], 1.0)
    ones_f = const.tile([1, DK], BF16, tag="onesf")
    nc.vector.memset(ones_f[:], 1.0)
    # warm up the exp table set early (one-time ~2.7us load)
    warm = const.tile([1, 2], F32, tag="warm")
    nc.vector.memset(warm[:], 0.0)
    nc.scalar.activation(warm[:], warm[:], Exp)

    # ---- resident inputs (d-interleaved so V proj / pair-0 proj can
    # start as soon as their first chunks land) ----
    xt_sb = xtp.tile([NP, NDC, T], BF16, tag="xt")
    wv_sb = wp.tile([NP, NDC, CD], BF16, tag="wv")
    wq_sb = wp.tile([NP, NDC, CD], BF16, tag="wq")
    wk_sb = wp.tile([NP, NDC, CD], BF16, tag="wk")
    wo_sb = wp.tile([NP, NPAIR, D], BF16, tag="wo")
    # first T-half of xt + wv unblock the upfront V tiles; pair-0 Wq/Wk
    # unblock its projections; everything else arrives behind them
    TH = T // 2
    for d in range(NDC):
        nc.sync.dma_start(xt_sb[:, d, 0:TH], xt[d * NP : (d + 1) * NP, 0:TH])
        nc.sync.dma_start(wv_sb[:, d, :], wv[d * NP : (d + 1) * NP, :])
    for d in range(NDC):
        nc.sync.dma_start(wq_sb[:, d, 0:NP], wq[d * NP : (d + 1) * NP, 0:NP])
        nc.sync.dma_start(wk_sb[:, d, 0:NP], wk[d * NP : (d + 1) * NP, 0:NP])
        nc.sync.dma_start(xt_sb[:, d, TH:T], xt[d * NP : (d + 1) * NP, TH:T])
    for c in range(1, NPAIR):
        for d in range(NDC):
            nc.sync.dma_start(
                wq_sb[:, d, c * NP : (c + 1) * NP],
                wq[d * NP : (d + 1) * NP, c * NP : (c + 1) * NP],
            )
            nc.sync.dma_start(
                wk_sb[:, d, c * NP : (c + 1) * NP],
                wk[d * NP : (d + 1) * NP, c * NP : (c + 1) * NP],
            )
    for c in range(NPAIR):
        nc.sync.dma_start(wo_sb[:, c, :], wo[c * NP : (c + 1) * NP, :])

    v_sb = [
        vpool.tile([NP, HL, DK + 1], BF16, tag=f"v{tt}", name=f"v{tt}")
        for tt in range(NTT)
    ]
    oat = [
        oatp.tile([NP, T], BF16, tag=f"oat{c}", name=f"oat{c}") for c in range(NPAIR)
    ]

    # ---- deferred-work emitters (injected between attention k-tiles) ----
    # deferred tasks are emitted in halves so injected PE work interleaves
    # finely with the attention stream; the PSUM chain tile is carried
    # between the two halves
    def emit_v(tt, half, carry):
        if half == 0:
            ps = projps.tile([NP, CD], F32, tag="proj", name="psv")
        else:
            ps = carry[tt]
        for d in range(4 * half, 4 * half + 4):
            nc.tensor.matmul(
                ps[:],
                xt_sb[:, d, tt * NP : (tt + 1) * NP],
                wv_sb[:, d, :],
                start=(d == 0),
                stop=False,
            )
        if half == 0:
            carry[tt] = ps
            return
        del carry[tt]
        # bias along free dim via K=1 rank-1 update
        nc.tensor.matmul(ps[:], ones_t[0:1, 0:NP], bv_sb[:], start=False, stop=True)
        vt = v_sb[tt]
        nc.vector.memset(vt[:, :, DK : DK + 1], 1.0)
        nc.vector.tensor_copy(vt[:, :, 0:DK], ps.rearrange("p (h k) -> p h k", h=HL))

    vcarry = {}

    def v_tasks(tts):
        for tt in tts:
            yield lambda tt=tt: emit_v(tt, 0, vcarry)
            yield lambda tt=tt: emit_v(tt, 1, vcarry)

    def emit_proj(dst, w_sb, b_sb, c, tb, half, carry):
        if half == 0:
            ps = projps.tile([NP, QB], F32, tag="proj", name="psp")
            carry[(c, tb, dst.name)] = ps
        else:
            ps = carry.pop((c, tb, dst.name))
        for d in range(4 * half, 4 * half + 4):
            nc.tensor.matmul(
                ps[:],
                w_sb[:, d, c * NP : (c + 1) * NP],
                xt_sb[:, d, tb * QB : (tb + 1) * QB],
                start=(d == 0),
                stop=(d == NDC - 1),
            )
        if half == 1:
            nc.vector.tensor_scalar_add(
                dst[:, tb * QB : (tb + 1) * QB], ps[:], b_sb[:, c : c + 1]
            )

    pcarry = {}

    def proj_tasks(c, qt, kt, tbs=range(NTB)):
        for tb in tbs:
            for half in range(2):
                yield lambda tb=tb, half=half: emit_proj(
                    qt, wq_sb, bq_sb, c, tb, half, pcarry
                )
            for half in range(2):
                yield lambda tb=tb, half=half: emit_proj(
                    kt, wk_sb, bk_sb, c, tb, half, pcarry
                )

    def final_task(tt, dh):
        ps = projps.tile([NP, QB], F32, tag="proj", name="psy")
        for cc in range(NPAIR):
            nc.tensor.matmul(
                ps[:],
                oat[cc][:, tt * NP : (tt + 1) * NP],
                wo_sb[:, cc, dh * QB : (dh + 1) * QB],
                start=(cc == 0),
                stop=(cc == NPAIR - 1),
            )
        ystage = smallp.tile([NP, QB], F32, tag="ystage", bufs=4, name="ystage")
        if (tt + dh) % 2 == 0:
            nc.vector.tensor_copy(ystage[:], ps[:])
        else:
            nc.scalar.copy(ystage[:], ps[:])
        eng = nc.sync if (tt + dh) % 2 == 0 else nc.gpsimd
        eng.dma_start(y[tt * NP : (tt + 1) * NP, dh * QB : (dh + 1) * QB], ystage[:])

    def norm_task(c, qb, se_q):
        # sumexp arrives partition-folded [128, 8]: reciprocal costs 8
        # elems/lane. Unfold to one partition-0 row, broadcast across 128
        # dv partitions via two concurrent col-tiled K=1 matmuls, then
        # normalize both heads' oat slice with a single in-place mul.
        rcp = smallp.tile([NP, 8], F32, tag="rcp", name="rcp")
        nc.vector.reciprocal(rcp[:], se_q[:])
        rcpb = smallp.tile([NP, 8], BF16, tag="rcpb", name="rcpb")
        nc.vector.tensor_copy(rcpb[:], rcp[:])
        rcprow = smallp.tile([1, 2 * QB], BF16, tag="rcprow", name="rcprow")
        for h in range(2):
            nc.gpsimd.dma_start(
                rcprow[0:1, h * QB : (h + 1) * QB], rcpb[:, h * 4 : (h + 1) * 4]
            )
        bc = projps.tile([NP, QB], F32, tag="proj", name="bc")
        for h in range(2):
            nc.tensor.matmul(
                bc[64 * h : 64 * h + 64, :],
                ones_f[0:1, 0:DK],
                rcprow[0:1, h * QB : (h + 1) * QB],
                start=True, stop=True,
                tile_position=(0, 64 * h),
            )
        sl = oat[c][:, qb * QB : (qb + 1) * QB]
        nc.vector.tensor_mul(sl, sl, bc[:])

    # ---- upfront: first V tiles + pair-0 first-half projections (all
    # depend only on the first T-half of xt) ----
    for t in v_tasks(range(NV_UP)):
        t()
    qt_c = qkp.tile([NP, T], BF16, tag="qt", name="qt0")
    kt_c = qkp.tile([NP, T], BF16, tag="kt", name="kt0")
    for t in proj_tasks(0, qt_c, kt_c, tbs=range(2)):
        t()

    # ---- per head-pair attention with injected deferred work ----
    pending = list(proj_tasks(0, qt_c, kt_c, tbs=range(2, NTB)))
    N_ITER = sum(4 * qb + 4 for qb in range(NTB))  # 40 k-tiles per pair
    for c in range(NPAIR):
        if c + 1 < NPAIR:
            qt_n = qkp.tile([NP, T], BF16, tag="qt", name=f"qt{c + 1}")
            kt_n = qkp.tile([NP, T], BF16, tag="kt", name=f"kt{c + 1}")
            if c == 0:
                pending += list(v_tasks(range(NV_UP, NTT)))
            pending += list(proj_tasks(c + 1, qt_n, kt_n))
        else:
            qt_n = kt_n = None
        # spread this pair's deferred work evenly over its 40 k-tiles
        # (pair 3 also absorbs per-qb norm + final-projection appends)
        est_tasks = len(pending) + (4 if c < NPAIR - 1 else 40)
        it = 0
        popped = 0

        for qb in range(NTB):
            nkt = 4 * qb + 4
            pv = [
                pvps.tile([DK + 1, QB], F32, tag=f"pv{h}", name=f"pv{h}")
                for h in range(2)
            ]
            for kti in range(nkt):
                while pending and popped < ((it + 1) * est_tasks) // N_ITER:
                    pending.pop(0)()
                    popped += 1
                it += 1
                di = kti - 4 * qb
                o = max(di, 0) * NP
                sp = scoreps.tile([NP, 2 * QB], F32, tag="s", name="sp")
                for h in range(2):
                    nc.tensor.matmul(
                        sp[:, h * QB + o : (h + 1) * QB],
                        kt_c[64 * h : 64 * h + 64, kti * NP : (kti + 1) * NP],
                        qt_c[64 * h : 64 * h + 64, qb * QB + o : (qb + 1) * QB],
                        start=True,
                        stop=True,
                        tile_position=(64 * h, 0),
                    )
                et = expp.tile([NP, 2 * QB], BF16, tag="e", name="et")
                nc.scalar.activation(et[:, o : 2 * QB], sp[:, o : 2 * QB], Exp, scale=0.125)
                if di >= 0:
                    ev = et.rearrange("p (g q) -> p g q", g=2)[:, :, o : o + NP]
                    nc.vector.tensor_mul(ev, ev, tri2_sb[:])
                for h in range(2):
                    nc.tensor.matmul(
                        pv[h][:, o:QB],
                        v_sb[kti][:, 2 * c + h, :],
                        et[:, h * QB + o : (h + 1) * QB],
                        start=(kti == 0),
                        stop=(kti == nkt - 1),
                    )
            se_q = smallp.tile([NP, 8], F32, tag="se", bufs=3, name=f"se{c}_{qb}")
            for h in range(2):
                # unnormalized out^T -> oat; sumexp row -> se_q partition-
                # folded [1,512]->[128,4] (same-partition staging copy +
                # cross-partition SBUF DMA off the sync queue)
                nc.vector.tensor_copy(
                    oat[c][64 * h : 64 * h + 64, qb * QB : (qb + 1) * QB],
                    pv[h][0:DK, :],
                )
                serow = smallp.tile([DK + 1, QB], F32, tag="serow", name="serow")
                nc.vector.tensor_copy(serow[DK : DK + 1, :], pv[h][DK : DK + 1, :])
                nc.gpsimd.dma_start(
                    se_q[:, h * 4 : (h + 1) * 4], serow[DK : DK + 1, :]
                )
            pending.append(lambda c=c, qb=qb, se_q=se_q: norm_task(c, qb, se_q))
            if c == NPAIR - 1:
                # final projection chunks for this qb's t-range become
                # injectable as soon as this qb's normalization runs
                # (earlier pairs' oat is long since normalized)
                for tt in range(4 * qb, 4 * qb + 4):
                    for dh in range(2):
                        pending.append(lambda tt=tt, dh=dh: final_task(tt, dh))
        qt_c, kt_c = qt_n, kt_n

    while pending:  # pair 3's tail: last normalizations + final chunks
        pending.pop(0)()


def _install_ntff_hook_shim():
    """The agent image's antenv lacks axon_hooks, so trace=True under axon
    degrades. Provide the missing module and register the ctypes NTFF hook
    from trn_agent_boot. Best-effort: failures just mean no trace."""
    try:
        import sys
        import types

        if "antenv.axon_hooks" not in sys.modules:
            mod = types.ModuleType("antenv.axon_hooks")
            mod._hook = None
            mod.set_axon_ntff_profile_hook = lambda h: setattr(mod, "_hook", h)
            mod.get_axon_ntff_profile_hook = lambda: mod._hook
            sys.modules["antenv.axon_hooks"] = mod
            import antenv

            antenv.axon_hooks = mod
        from antenv.axon_hooks import (
            get_axon_ntff_profile_hook,
            set_axon_ntff_profile_hook,
        )

        if get_axon_ntff_profile_hook() is None:
            from trn_agent_boot.trn_boot import _ntff_profile_via_ctypes

            hook = _ntff_profile_via_ctypes("/opt/axon/libaxon_pjrt.so")
            if hook is not None:
                set_axon_ntff_profile_hook(hook)
    except Exception as e:  # noqa: BLE001
        print(f"ntff hook shim failed ({e}); running without trace")


def _bf(a: np.ndarray) -> np.ndarray:
    return np.ascontiguousarray(a, dtype=np.float32).astype(ml_dtypes.bfloat16)


def _make_tri() -> np.ndarray:
    # tri[r, j] = 1 if j >= r else 0 (valid region of a boundary tile)
    r = np.arange(NP)[:, None]
    j = np.arange(NP)[None, :]
    return (j >= r).astype(ml_dtypes.bfloat16)


def kernel(x, Wq, bq, Wk, bk, Wv, bv, Wo, bo):
    x = np.ascontiguousarray(np.asarray(x, dtype=np.float32))
    Wq, bq = np.asarray(Wq, np.float32), np.asarray(bq, np.float32)
    Wk, bk = np.asarray(Wk, np.float32), np.asarray(bk, np.float32)
    Wv, bv = np.asarray(Wv, np.float32), np.asarray(bv, np.float32)
    Wo, bo = np.asarray(Wo, np.float32), np.asarray(bo, np.float32)

    if "nc" not in _CACHE:
        _CACHE["nc"] = _build_nc()
    nc = _CACHE["nc"]

    tri = _make_tri()
    in_maps = []
    for core in range(8):
        b, hg = core // 2, core % 2
        cs = slice(hg * CD, (hg + 1) * CD)
        in_maps.append(
            {
                "xt": _bf(x[b].T),
                "wq": _bf(Wq[:, cs]),
                "wk": _bf(Wk[:, cs]),
                "wv": _bf(Wv[:, cs]),
                "wo": _bf(Wo[cs, :]),
                "bqc": np.ascontiguousarray(bq[cs].reshape(NPAIR, NP).T),
                "bkc": np.ascontiguousarray(bk[cs].reshape(NPAIR, NP).T),
                "bvr": _bf(bv[cs].reshape(1, CD)),
                "trid": tri,
            }
        )

    trace = bool(os.environ.get("KERNEL_TRACE"))
    if trace:
        _install_ntff_hook_shim()
    res = run_bass_kernel_spmd(nc, in_maps, core_ids=list(range(8)), trace=trace)
    _CACHE["last_results"] = res

    out = np.empty((B, T, D), dtype=np.float32)
    for b in range(B):
        out[b] = res.results[2 * b]["y"] + res.results[2 * b + 1]["y"] + bo
    return out


# revision 24
# speedup vs baseline: 1.2852x; 1.0298x over previous
"""Causal self-attention (B=4, T=2048, D=1024, H=16) on 8 NeuronCores.

Sharding: core c handles batch b=c//2 and head-group hg=c%2 (8 of 16 heads).
Per core: column-parallel Wq/Wk/Wv (512 cols), row-parallel Wo (512 rows).
Host sums the two partial outputs per batch and adds bo. No collectives.

All-bf16 compute (f32 PSUM accumulation). On-chip layout (transposed):
  xt [D=1024, T=2048] resident in SBUF (loaded once), qT/kT [128, T] per
  head pair, V natural [tk, 8 heads x (64 dv + 1 ones col)].
  S^T [tk, tq] for BOTH heads of a pair lands in one 2-bank PSUM tile
  [128, 1024] via two concurrent row-tiled matmuls; ONE exp per k-tile
  covers both heads (no max subtraction -- scores are O(4), exp safe).
  Causal masking: diagonal k-tiles narrow matmul/exp to the valid column
  range; the triangular boundary is zeroed AFTER exp by a [128,128] bf16
  multiply (same result as -inf masking, off the scalar-engine path).
  PV matmul out^T[dv,tq] = V_aug.T @ expS^T; the ones column yields
  sumexp for free. Sumexp rows are gathered cross-partition into one
  [8,512] tile per pair (DVE copy + tiny SBUF->SBUF DMA) so reciprocal
  runs batched (the per-row [1,512] reciprocal was 3.3us each).
  Normalization: K=1 broadcast matmul of 1/sumexp + in-place DVE mul
  into oat; final projection consumes oat directly as lhsT.

PE (HAM) warmth: the scalar engine's exp is the attention-phase pace
setter, so next pair's Q/K projection chains and deferred V-projection
tiles are emitted interleaved between attention k-tiles to keep the
tensor engine dense (K=8/8 clock).
"""

import os
from contextlib import ExitStack

import ml_dtypes
import numpy as np

import concourse.bacc as bacc
import concourse.mybir as mybir
import concourse.tile as tile
from concourse.bass_utils import run_bass_kernel_spmd

B, T, D, H, DK = 4, 2048, 1024, 16, 64
HL = 8  # heads per core
CD = HL * DK  # 512 local channels
NP = 128  # partitions
QB = 512  # query block
NDC = D // NP  # 8 din chunks
NTT = T // NP  # 16 t-tiles
NTB = T // QB  # 4 t-blocks
NPAIR = HL // 2  # 4 head pairs
NV_UP = 5  # V tiles emitted upfront (rest injected into attention 0)
F32 = mybir.dt.float32
BF16 = mybir.dt.bfloat16
Exp = mybir.ActivationFunctionType.Exp

_CACHE: dict = {}


def _build_nc():
    nc = bacc.Bacc("TRN2", target_bir_lowering=False, debug=False)
    xt = nc.dram_tensor("xt", [D, T], BF16, kind="ExternalInput")
    wq = nc.dram_tensor("wq", [D, CD], BF16, kind="ExternalInput")
    wk = nc.dram_tensor("wk", [D, CD], BF16, kind="ExternalInput")
    wv = nc.dram_tensor("wv", [D, CD], BF16, kind="ExternalInput")
    wo = nc.dram_tensor("wo", [CD, D], BF16, kind="ExternalInput")
    bqc = nc.dram_tensor("bqc", [NP, NPAIR], F32, kind="ExternalInput")
    bkc = nc.dram_tensor("bkc", [NP, NPAIR], F32, kind="ExternalInput")
    bvr = nc.dram_tensor("bvr", [1, CD], BF16, kind="ExternalInput")
    trid = nc.dram_tensor("trid", [NP, NP], BF16, kind="ExternalInput")
    y = nc.dram_tensor("y", [T, D], F32, kind="ExternalOutput")

    with tile.TileContext(nc) as tc, ExitStack() as ctx:
        _body(nc, tc, ctx, xt, wq, wk, wv, wo, bqc, bkc, bvr, trid, y)
    nc.compile()
    return nc


def _body(nc, tc, ctx, xt, wq, wk, wv, wo, bqc, bkc, bvr, trid, y):
    const = ctx.enter_context(tc.tile_pool(name="const", bufs=1))
    xtp = ctx.enter_context(tc.tile_pool(name="xtp", bufs=1))
    wp = ctx.enter_context(tc.tile_pool(name="wp", bufs=1))
    vpool = ctx.enter_context(tc.tile_pool(name="v", bufs=1))
    oatp = ctx.enter_context(tc.tile_pool(name="oat", bufs=1))
    qkp = ctx.enter_context(tc.tile_pool(name="qk", bufs=1))
    expp = ctx.enter_context(tc.tile_pool(name="exp", bufs=3))
    smallp = ctx.enter_context(tc.tile_pool(name="small", bufs=2))
    # PSUM: proj 2 + score (2-bank x 2) 4 + pv0 1 + pv1 1 = 8 banks
    projps = ctx.enter_context(tc.tile_pool(name="projps", bufs=2, space="PSUM"))
    scoreps = ctx.enter_context(tc.tile_pool(name="scoreps", bufs=2, space="PSUM"))
    pvps = ctx.enter_context(tc.tile_pool(name="pvps", bufs=1, space="PSUM"))

    # ---- constants ----
    tri2_sb = const.tile([NP, 2, NP], BF16, tag="tri")
    for g in range(2):
        nc.sync.dma_start(tri2_sb[:, g, :], trid[:])
    bq_sb = const.tile([NP, NPAIR], F32, tag="bq")
    nc.sync.dma_start(bq_sb[:], bqc[:])
    bk_sb = const.tile([NP, NPAIR], F32, tag="bk")
    nc.sync.dma_start(bk_sb[:], bkc[:])
    bv_sb = const.tile([1, CD], BF16, tag="bv")
    nc.sync.dma_start(bv_sb[:], bvr[:])
    ones_t = const.tile([1, NP], BF16, tag="onest")
    nc.vector.memset(ones_t[:], 1.0)
    ones_f = const.tile([1, DK], BF16, tag="onesf")
    nc.vector.memset(ones_f[:], 1.0)
    # warm up the exp table set early (one-time ~2.7us load)
    warm = const.tile([1, 2], F32, tag="warm")
    nc.vector.memset(warm[:], 0.0)
    nc.scalar.activation(warm[:], warm[:], Exp)

    # ---- resident inputs (d-interleaved so V proj / pair-0 proj can
    # start as soon as their first chunks land) ----
    xt_sb = xtp.tile([NP, NDC, T], BF16, tag="xt")
    wv_sb = wp.tile([NP, NDC, CD], BF16, tag="wv")
    wq_sb = wp.tile([NP, NDC, CD], BF16, tag="wq")
    wk_sb = wp.tile([NP, NDC, CD], BF16, tag="wk")
    wo_sb = wp.tile([NP, NPAIR, D], BF16, tag="wo")
    # first T-half of xt + wv unblock the upfront V tiles; pair-0 Wq/Wk
    # unblock its projections; everything else arrives behind them
    TH = T // 2
    for d in range(NDC):
        nc.sync.dma_start(xt_sb[:, d, 0:TH], xt[d * NP : (d + 1) * NP, 0:TH])
        nc.sync.dma_start(wv_sb[:, d, :], wv[d * NP : (d + 1) * NP, :])
    for d in range(NDC):
        nc.sync.dma_start(wq_sb[:, d, 0:NP], wq[d * NP : (d + 1) * NP, 0:NP])
        nc.sync.dma_start(wk_sb[:, d, 0:NP], wk[d * NP : (d + 1) * NP, 0:NP])
        nc.sync.dma_start(xt_sb[:, d, TH:T], xt[d * NP : (d + 1) * NP, TH:T])
    for c in range(1, NPAIR):
        for d in range(NDC):
            nc.sync.dma_start(
                wq_sb[:, d, c * NP : (c + 1) * NP],
                wq[d * NP : (d + 1) * NP, c * NP : (c + 1) * NP],
            )
            nc.sync.dma_start(
                wk_sb[:, d, c * NP : (c + 1) * NP],
                wk[d * NP : (d + 1) * NP, c * NP : (c + 1) * NP],
            )
    for c in range(NPAIR):
        nc.sync.dma_start(wo_sb[:, c, :], wo[c * NP : (c + 1) * NP, :])

    v_sb = [
        vpool.tile([NP, HL, DK + 1], BF16, tag=f"v{tt}", name=f"v{tt}")
        for tt in range(NTT)
    ]
    oat = [
        oatp.tile([NP, T], BF16, tag=f"oat{c}", name=f"oat{c}") for c in range(NPAIR)
    ]

    # ---- deferred-work emitters (injected between attention k-tiles) ----
    # deferred tasks are emitted in halves so injected PE work interleaves
    # finely with the attention stream; the PSUM chain tile is carried
    # between the two halves
    def emit_v(tt, half, carry):
        if half == 0:
            ps = projps.tile([NP, CD], F32, tag="proj", name="psv")
        else:
            ps = carry[tt]
        for d in range(4 * half, 4 * half + 4):
            nc.tensor.matmul(
                ps[:],
                xt_sb[:, d, tt * NP : (tt + 1) * NP],
                wv_sb[:, d, :],
                start=(d == 0),
                stop=False,
            )
        if half == 0:
            carry[tt] = ps
            return
        del carry[tt]
        # bias along free dim via K=1 rank-1 update
        nc.tensor.matmul(ps[:], ones_t[0:1, 0:NP], bv_sb[:], start=False, stop=True)
        vt = v_sb[tt]
        nc.vector.memset(vt[:, :, DK : DK + 1], 1.0)
        nc.vector.tensor_copy(vt[:, :, 0:DK], ps.rearrange("p (h k) -> p h k", h=HL))

    vcarry = {}

    def v_tasks(tts):
        for tt in tts:
            yield lambda tt=tt: emit_v(tt, 0, vcarry)
            yield lambda tt=tt: emit_v(tt, 1, vcarry)

    def emit_proj(dst, w_sb, b_sb, c, tb, half, carry):
        if half == 0:
            ps = projps.tile([NP, QB], F32, tag="proj", name="psp")
            carry[(c, tb, dst.name)] = ps
        else:
            ps = carry.pop((c, tb, dst.name))
        for d in range(4 * half, 4 * half + 4):
            nc.tensor.matmul(
                ps[:],
                w_sb[:, d, c * NP : (c + 1) * NP],
                xt_sb[:, d, tb * QB : (tb + 1) * QB],
                start=(d == 0),
                stop=(d == NDC - 1),
            )
        if half == 1:
            nc.vector.tensor_scalar_add(
                dst[:, tb * QB : (tb + 1) * QB], ps[:], b_sb[:, c : c + 1]
            )

    pcarry = {}

    def proj_tasks(c, qt, kt, tbs=range(NTB)):
        for tb in tbs:
            for half in range(2):
                yield lambda tb=tb, half=half: emit_proj(
                    qt, wq_sb, bq_sb, c, tb, half, pcarry
                )
            for half in range(2):
                yield lambda tb=tb, half=half: emit_proj(
                    kt, wk_sb, bk_sb, c, tb, half, pcarry
                )

    def final_task(tt, dh):
        ps = projps.tile([NP, QB], F32, tag="proj", name="psy")
        for cc in range(NPAIR):
            nc.tensor.matmul(
                ps[:],
                oat[cc][:, tt * NP : (tt + 1) * NP],
                wo_sb[:, cc, dh * QB : (dh + 1) * QB],
                start=(cc == 0),
                stop=(cc == NPAIR - 1),
            )
        ystage = smallp.tile([NP, QB], F32, tag="ystage", bufs=4, name="ystage")
        if (tt + dh) % 2 == 0:
            nc.vector.tensor_copy(ystage[:], ps[:])
        else:
            nc.scalar.copy(ystage[:], ps[:])
        eng = nc.sync if (tt + dh) % 2 == 0 else nc.gpsimd
        eng.dma_start(y[tt * NP : (tt + 1) * NP, dh * QB : (dh + 1) * QB], ystage[:])

    def norm_task(c, qb, se_q):
        # sumexp arrives partition-folded [128, 8]: reciprocal costs 8
        # elems/lane. Unfold to one partition-0 row, broadcast across 128
        # dv partitions via two concurrent col-tiled K=1 matmuls, then
        # normalize both heads' oat slice with a single in-place mul.
        rcp = smallp.tile([NP, 8], F32, tag="rcp", name="rcp")
        nc.vector.reciprocal(rcp[:], se_q[:])
        rcpb = smallp.tile([NP, 8], BF16, tag="rcpb", name="rcpb")
        nc.vector.tensor_copy(rcpb[:], rcp[:])
        rcprow = smallp.tile([1, 2 * QB], BF16, tag="rcprow", name="rcprow")
        for h in range(2):
            nc.gpsimd.dma_start(
                rcprow[0:1, h * QB : (h + 1) * QB], rcpb[:, h * 4 : (h + 1) * 4]
            )
        bc = projps.tile([NP, QB], F32, tag="proj", name="bc")
        for h in range(2):
            nc.tensor.matmul(
                bc[64 * h : 64 * h + 64, :],
                ones_f[0:1, 0:DK],
                rcprow[0:1, h * QB : (h + 1) * QB],
                start=True, stop=True,
                tile_position=(0, 64 * h),
            )
        sl = oat[c][:, qb * QB : (qb + 1) * QB]
        nc.vector.tensor_mul(sl, sl, bc[:])

    # ---- upfront: first V tiles + every pair's tb0 projections (all
    # depend only on the first T-half of xt) ----
    for t in v_tasks(range(NV_UP)):
        t()
    qts = [qkp.tile([NP, T], BF16, tag=f"qt{c}", name=f"qt{c}") for c in range(NPAIR)]
    kts = [qkp.tile([NP, T], BF16, tag=f"kt{c}", name=f"kt{c}") for c in range(NPAIR)]
    for c in range(NPAIR):
        for t in proj_tasks(c, qts[c], kts[c], tbs=(0,)):
            t()

    # ---- attention qb-outer / pair-inner: block density increases
    # monotonically so the PE clock gate stays warm; deferred work is
    # queued in deadline order and paced uniformly over the k-tiles ----
    pending = []
    for tt in range(NV_UP, 8):
        pending += list(v_tasks((tt,)))
    for c in range(NPAIR):
        pending += list(proj_tasks(c, qts[c], kts[c], tbs=(1,)))
    for tt in range(8, 12):
        pending += list(v_tasks((tt,)))
    for c in range(NPAIR):
        pending += list(proj_tasks(c, qts[c], kts[c], tbs=(2,)))
    for tt in range(12, NTT):
        pending += list(v_tasks((tt,)))
    for c in range(NPAIR):
        pending += list(proj_tasks(c, qts[c], kts[c], tbs=(3,)))

    N_ITER = NPAIR * sum(4 * qb + 4 for qb in range(NTB))  # 160 k-tiles
    EST_TOTAL = len(pending) + 16 + 32  # + norm and final appends
    git = 0
    popped = 0
    for qb in range(NTB):
        for c in range(NPAIR):
            qt_c, kt_c = qts[c], kts[c]
            nkt = 4 * qb + 4
            pv = [
                pvps.tile([DK + 1, QB], F32, tag=f"pv{h}", name=f"pv{h}")
                for h in range(2)
            ]
            for kti in range(nkt):
                while pending and popped < ((git + 1) * EST_TOTAL) // N_ITER:
                    pending.pop(0)()
                    popped += 1
                git += 1
                di = kti - 4 * qb
                o = max(di, 0) * NP
                sp = scoreps.tile([NP, 2 * QB], F32, tag="s", name="sp")
                for h in range(2):
                    nc.tensor.matmul(
                        sp[:, h * QB + o : (h + 1) * QB],
                        kt_c[64 * h : 64 * h + 64, kti * NP : (kti + 1) * NP],
                        qt_c[64 * h : 64 * h + 64, qb * QB + o : (qb + 1) * QB],
                        start=True,
                        stop=True,
                        tile_position=(64 * h, 0),
                    )
                et = expp.tile([NP, 2 * QB], BF16, tag="e", name="et")
                nc.scalar.activation(et[:, o : 2 * QB], sp[:, o : 2 * QB], Exp, scale=0.125)
                if di >= 0:
                    ev = et.rearrange("p (g q) -> p g q", g=2)[:, :, o : o + NP]
                    nc.vector.tensor_mul(ev, ev, tri2_sb[:])
                for h in range(2):
                    nc.tensor.matmul(
                        pv[h][:, o:QB],
                        v_sb[kti][:, 2 * c + h, :],
                        et[:, h * QB + o : (h + 1) * QB],
                        start=(kti == 0),
                        stop=(kti == nkt - 1),
                    )
            se_q = smallp.tile([NP, 8], F32, tag="se", bufs=3, name=f"se{c}_{qb}")
            for h in range(2):
                # unnormalized out^T -> oat; sumexp row -> se_q partition-
                # folded [1,512]->[128,4] (same-partition staging copy +
                # cross-partition SBUF DMA off the sync queue)
                nc.vector.tensor_copy(
                    oat[c][64 * h : 64 * h + 64, qb * QB : (qb + 1) * QB],
                    pv[h][0:DK, :],
                )
                serow = smallp.tile([DK + 1, QB], F32, tag="serow", name="serow")
                nc.vector.tensor_copy(serow[DK : DK + 1, :], pv[h][DK : DK + 1, :])
                nc.gpsimd.dma_start(
                    se_q[:, h * 4 : (h + 1) * 4], serow[DK : DK + 1, :]
                )
            pending.append(lambda c=c, qb=qb, se_q=se_q: norm_task(c, qb, se_q))
            if c == NPAIR - 1:
                # final projection chunks for this qb's t-range become
                # injectable once all four pairs' norms for it are queued
                for tt in range(4 * qb, 4 * qb + 4):
                    for dh in range(2):
                        pending.append(lambda tt=tt, dh=dh: final_task(tt, dh))

    while pending:  # tail: last normalizations + final chunks
        pending.pop(0)()


def _install_ntff_hook_shim():
    """The agent image's antenv lacks axon_hooks, so trace=True under axon
    degrades. Provide the missing module and register the ctypes NTFF hook
    from trn_agent_boot. Best-effort: failures just mean no trace."""
    try:
        import sys
        import types

        if "antenv.axon_hooks" not in sys.modules:
            mod = types.ModuleType("antenv.axon_hooks")
            mod._hook = None
            mod.set_axon_ntff_profile_hook = lambda h: setattr(mod, "_hook", h)
            mod.get_axon_ntff_profile_hook = lambda: mod._hook
            sys.modules["antenv.axon_hooks"] = mod
            import antenv

            antenv.axon_hooks = mod
        from antenv.axon_hooks import (
            get_axon_ntff_profile_hook,
            set_axon_ntff_profile_hook,
        )

        if get_axon_ntff_profile_hook() is None:
            from trn_agent_boot.trn_boot import _ntff_profile_via_ctypes

            hook = _ntff_profile_via_ctypes("/opt/axon/libaxon_pjrt.so")
            if hook is not None:
                set_axon_ntff_profile_hook(hook)
    except Exception as e:  # noqa: BLE001
        print(f"ntff hook shim failed ({e}); running without trace")


def _bf(a: np.ndarray) -> np.ndarray:
    return np.ascontiguousarray(a, dtype=np.float32).astype(ml_dtypes.bfloat16)


def _make_tri() -> np.ndarray:
    # tri[r, j] = 1 if j >= r else 0 (valid region of a boundary tile)
    r = np.arange(NP)[:, None]
    j = np.arange(NP)[None, :]
    return (j >= r).astype(ml_dtypes.bfloat16)


def kernel(x, Wq, bq, Wk, bk, Wv, bv, Wo, bo):
    x = np.ascontiguousarray(np.asarray(x, dtype=np.float32))
    Wq, bq = np.asarray(Wq, np.float32), np.asarray(bq, np.float32)
    Wk, bk = np.asarray(Wk, np.float32), np.asarray(bk, np.float32)
    Wv, bv = np.asarray(Wv, np.float32), np.asarray(bv, np.float32)
    Wo, bo = np.asarray(Wo, np.float32), np.asarray(bo, np.float32)

    if "nc" not in _CACHE:
        _CACHE["nc"] = _build_nc()
    nc = _CACHE["nc"]

    tri = _make_tri()
    in_maps = []
    for core in range(8):
        b, hg = core // 2, core % 2
        cs = slice(hg * CD, (hg + 1) * CD)
        in_maps.append(
            {
                "xt": _bf(x[b].T),
                "wq": _bf(Wq[:, cs]),
                "wk": _bf(Wk[:, cs]),
                "wv": _bf(Wv[:, cs]),
                "wo": _bf(Wo[cs, :]),
                "bqc": np.ascontiguousarray(bq[cs].reshape(NPAIR, NP).T),
                "bkc": np.ascontiguousarray(bk[cs].reshape(NPAIR, NP).T),
                "bvr": _bf(bv[cs].reshape(1, CD)),
                "trid": tri,
            }
        )

    trace = bool(os.environ.get("KERNEL_TRACE"))
    if trace:
        _install_ntff_hook_shim()
    res = run_bass_kernel_spmd(nc, in_maps, core_ids=list(range(8)), trace=trace)
    _CACHE["last_results"] = res

    out = np.empty((B, T, D), dtype=np.float32)
    for b in range(B):
        out[b] = res.results[2 * b]["y"] + res.results[2 * b + 1]["y"] + bo
    return out
